# revision 1
# baseline (speedup 1.0000x reference)
"""MHA + RoPE fused kernel for Trainium2, sharded tensor-parallel over heads
across 8 NeuronCores.

Problem (hardcoded): B=4, S=2048, E=1024, H=16 heads, D=64.
  xq = x @ wq.T ; xk = x @ wk.T ; xv = x @ wv.T          [B,S,H,D]
  RoPE(xq, xk) with angles dt[b,s] * inv_freq[r]
  scores = softmax(xq @ xk.T / sqrt(D) + mask)            per (b, head)
  out = (scores @ xv) reshaped to [B,S,E]; y = out @ wo.T + bo

Sharding: each core owns 2 heads (128 channels of q/k/v) and the matching
128 rows of wo.T; it computes a full partial y (row-parallel output
projection) and the host sums the 8 partials (the "all-reduce" on host).

Device layouts (per core):
  xT   [E=1024, T=8192]  bf16  (host-transposed x, shared by all cores)
  qT/kT [128, T] on SBUF, channel rows permuted per head to
        [even-freqs(32) | odd-freqs(32)] so RoPE pairs are 32-row blocks.
  scoresT[j, i] per (head, batch): keys on partitions -> softmax exp on
        ScalarE (bias arg carries the key-padding mask), column sums via a
        ones-column appended to V in the attn@V matmul (M=65).
  attn-out [d2=128, T] -> output projection vs woT tiles -> yT [1024, T].
"""

import sys

sys.path.insert(0, "/opt/trn_rl_repo")

import numpy as np
import ml_dtypes

import concourse.bass as bass
from concourse import bacc
import concourse.tile as tile
from concourse import mybir
from concourse.bass_utils import run_bass_kernel_spmd

F32 = mybir.dt.float32
BF16 = mybir.dt.bfloat16

B, S, E, H, D = 4, 2048, 1024, 16, 64
T = B * S                      # 8192 flattened tokens
NCORES = 8
HPC = H // NCORES              # 2 heads per core
CPC = HPC * D                  # 128 channels per core
NCHUNK = T // 512              # 16 phase-1 t-chunks
KT = E // 128                  # 8 contraction tiles
THETA = 10000.0
NEG_INF = -1e30

_prog_cache = {}


def _build_program(use_mask: bool):
    """One Bass program, identical on every core (data differs per core)."""
    nc = bacc.Bacc()

    xT_d = nc.dram_tensor("xT", [E, T], BF16, kind="ExternalInput")
    cc_d = nc.dram_tensor("cc", [128, T], F32, kind="ExternalInput")
    ss_d = nc.dram_tensor("ss", [128, T], F32, kind="ExternalInput")
    wq_d = nc.dram_tensor("wqT", [E, CPC], BF16, kind="ExternalInput")
    wk_d = nc.dram_tensor("wkT", [E, CPC], BF16, kind="ExternalInput")
    wv_d = nc.dram_tensor("wvT", [E, CPC], BF16, kind="ExternalInput")
    wo_d = nc.dram_tensor("woT", [CPC, E], BF16, kind="ExternalInput")
    scr_d = nc.dram_tensor("csscr", [B * 4, 1024], F32)  # colsum-recip bounce rows
    mb_d = None
    if use_mask:
        mb_d = nc.dram_tensor("mb", [128, B * 16], F32, kind="ExternalInput")
    y_d = nc.dram_tensor("yT", [E, T], F32, kind="ExternalOutput")

    xT_r = xT_d.rearrange("(k p) t -> p k t", p=128)
    wq_r = wq_d.rearrange("(k p) c -> p k c", p=128)
    wk_r = wk_d.rearrange("(k p) c -> p k c", p=128)
    wv_r = wv_d.rearrange("(k p) c -> p k c", p=128)
    wo_r = wo_d.rearrange("p (k c) -> p k c", c=128)

    with tile.TileContext(nc) as tc:
        with (
            tc.tile_pool(name="consts", bufs=1) as consts,
            tc.tile_pool(name="big", bufs=1) as big,
            tc.tile_pool(name="ph1", bufs=4) as ph1,
            tc.tile_pool(name="rope", bufs=2) as rope,
            tc.tile_pool(name="pt", bufs=2) as ptp,
            tc.tile_pool(name="norm", bufs=2) as norm,
            tc.tile_pool(name="ph3", bufs=3) as ph3,
            tc.tile_pool(name="psS", bufs=2, space="PSUM") as psS,
            tc.tile_pool(name="psO", bufs=2, space="PSUM") as psO,
        ):
            # ---- constants ----
            wq_sb = consts.tile([128, KT, CPC], BF16)
            wk_sb = consts.tile([128, KT, CPC], BF16)
            wv_sb = consts.tile([128, KT, CPC], BF16)
            wo_sb = consts.tile([128, KT, 128], BF16)
            nc.sync.dma_start(wq_sb, wq_r)
            nc.sync.dma_start(wk_sb, wk_r)
            nc.sync.dma_start(wv_sb, wv_r)
            nc.sync.dma_start(wo_sb, wo_r)
            mb_sb = None
            if use_mask:
                mb_sb = consts.tile([128, B * 16], F32)
                nc.sync.dma_start(mb_sb, mb_d[:, :])

            # ---- persistent activations ----
            qT_sb = big.tile([128, NCHUNK, 512], BF16)
            kT_sb = big.tile([128, NCHUNK, 512], BF16)
            vA_sb = big.tile([128, T // 128, 65], BF16)
            vB_sb = big.tile([128, T // 128, 65], BF16)
            attnT_sb = big.tile([128, NCHUNK, 512], BF16)
            nc.vector.memset(vA_sb[:, :, 64], 1.0)
            nc.vector.memset(vB_sb[:, :, 64], 1.0)

            def phase1_load(ch):
                xsb = ph1.tile([128, KT, 512], BF16, tag="xsb")
                for k in range(KT):   # one DMA per k-tile -> spread queues
                    nc.sync.dma_start(xsb[:, k, :],
                                      xT_r[:, k, ch * 512:(ch + 1) * 512])
                cc_sb = ph1.tile([128, 512], F32, tag="cc")
                ss_sb = ph1.tile([128, 512], F32, tag="ss")
                for h0 in (0, 256):
                    nc.sync.dma_start(cc_sb[:, h0:h0 + 256],
                                      cc_d[:, ch * 512 + h0:ch * 512 + h0 + 256])
                    nc.sync.dma_start(ss_sb[:, h0:h0 + 256],
                                      ss_d[:, ch * 512 + h0:ch * 512 + h0 + 256])
                return xsb, cc_sb, ss_sb

            def phase1_qk(ch, tiles):
                xsb, cc_sb, ss_sb = tiles
                for name, w_sb, dstT in (("q", wq_sb, qT_sb), ("k", wk_sb, kT_sb)):
                    ps = psS.tile([128, 512], F32, tag="sc", name="ps_qk")
                    for k in range(KT):
                        nc.tensor.matmul(ps, w_sb[:, k, :], xsb[:, k, :],
                                         start=(k == 0), stop=(k == KT - 1))
                    t1 = rope.tile([128, 512], F32, tag="t1")
                    t2 = rope.tile([128, 512], F32, tag="t2")
                    t2sw = rope.tile([128, 512], F32, tag="t2sw")
                    nc.vector.tensor_tensor(t1, ps, cc_sb, mybir.AluOpType.mult)
                    nc.vector.tensor_tensor(t2, ps, ss_sb, mybir.AluOpType.mult)
                    for b0 in (0, 64):
                        nc.sync.dma_start(t2sw[b0:b0 + 32], t2[b0 + 32:b0 + 64])
                        nc.sync.dma_start(t2sw[b0 + 32:b0 + 64], t2[b0:b0 + 32])
                    nc.vector.tensor_tensor(dstT[:, ch, :], t1, t2sw,
                                            mybir.AluOpType.add)

            def phase1_v(ch, tiles):
                xsb, _, _ = tiles
                for tt in range(4):
                    psv = psO.tile([128, 128], F32, tag="out", name="psv")
                    for k in range(KT):
                        nc.tensor.matmul(psv, xsb[:, k, tt * 128:(tt + 1) * 128],
                                         wv_sb[:, k, :],
                                         start=(k == 0), stop=(k == KT - 1))
                    ti = ch * 4 + tt
                    nc.vector.tensor_copy(vA_sb[:, ti, 0:64], psv[:, 0:64])
                    nc.vector.tensor_copy(vB_sb[:, ti, 0:64], psv[:, 64:128])

            def phase2_ih(b, ih, fillers=()):
                fillers = list(fillers)
                if True:                     # i-half of 1024 tokens
                    pos = [psO.tile([128, 1024], F32, tag="out", name=f"po{_h}")
                           for _h in range(2)]
                    for jb in range(16):
                        pss = [psS.tile([128, 1024], F32, tag="sc", name=f"sc{_h}")
                               for _h in range(2)]
                        ch_j = b * 4 + jb // 4
                        off_j = (jb % 4) * 128
                        for i2 in range(2):
                            ch_i = b * 4 + ih * 2 + i2
                            for hh, b0 in ((0, 0), (1, 64)):
                                nc.tensor.matmul(
                                    pss[hh][:, i2 * 512:(i2 + 1) * 512],
                                    kT_sb[b0:b0 + 64, ch_j, off_j:off_j + 128],
                                    qT_sb[b0:b0 + 64, ch_i, :],
                                    start=True, stop=True,
                                    tile_position=(b0, 0))
                        pT = [ptp.tile([128, 1024], BF16, tag=f"pT{_h}",
                                       name=f"pT{_h}") for _h in range(2)]
                        for hh in range(2):
                            bias = mb_sb[:, b * 16 + jb:b * 16 + jb + 1] if use_mask else 0.0
                            nc.scalar.activation(
                                pT[hh], pss[hh],
                                mybir.ActivationFunctionType.Exp,
                                bias=bias, scale=0.125)
                        for i2 in range(2):
                            for hh, v_sb in ((0, vA_sb), (1, vB_sb)):
                                nc.tensor.matmul(
                                    pos[hh][0:65, i2 * 512:(i2 + 1) * 512],
                                    v_sb[:, b * 16 + jb, :],
                                    pT[hh][:, i2 * 512:(i2 + 1) * 512],
                                    start=(jb == 0), stop=(jb == 15))
                        if jb % 4 == 3 and fillers:
                            fillers.pop(0)()
                    for hh in range(2):
                        # copy out of PSUM promptly so the out-slots free up
                        ocp = norm.tile([64, 1024], F32, tag="ocp")
                        nc.vector.tensor_copy(ocp, pos[hh][0:64, :])
                        csrow = norm.tile([1, 1024], F32, tag="csrow")
                        nc.vector.tensor_copy(csrow, pos[hh][64:65, :])
                        cs_rec = norm.tile([1, 1024], F32, tag="csrec")
                        nc.vector.reciprocal_approx_fast(out=cs_rec, in_=csrow)
                        srow = (b * 2 + ih) * 2 + hh
                        nc.sync.dma_start(scr_d[srow:srow + 1, :], cs_rec)
                        csrep = norm.tile([64, 1024], F32, tag="csrep")
                        nc.gpsimd.dma_start(
                            csrep, scr_d[srow, :].partition_broadcast(64))
                        for i2 in range(2):
                            ch_i = b * 4 + ih * 2 + i2
                            nc.vector.tensor_tensor(
                                attnT_sb[hh * 64:hh * 64 + 64, ch_i, :],
                                ocp[:, i2 * 512:(i2 + 1) * 512],
                                csrep[:, i2 * 512:(i2 + 1) * 512],
                                mybir.AluOpType.mult)

            def phase3_batch(b):
                for cck in range(KT):
                    for tc4 in range(4):
                        ch = b * 4 + tc4
                        psy = psO.tile([128, 512], F32, tag="out", name="psy")
                        nc.tensor.matmul(psy, wo_sb[:, cck, :], attnT_sb[:, ch, :],
                                         start=True, stop=True)
                        ysb = ph3.tile([128, 512], F32, tag="ysb")
                        nc.vector.tensor_copy(ysb, psy)
                        nc.sync.dma_start(
                            y_d[cck * 128:(cck + 1) * 128,
                                ch * 512:(ch + 1) * 512], ysb)

            tiles0 = [phase1_load(ch) for ch in range(4)]
            for ch in range(4):
                phase1_qk(ch, tiles0[ch])
                phase1_v(ch, tiles0[ch])
            for b in range(B):
                fillers = []
                tiles_next = None
                if b + 1 < B:
                    tiles_next = [phase1_load((b + 1) * 4 + i) for i in range(4)]
                    fillers = [
                        (lambda c=(b + 1) * 4 + i, t=tiles_next[i]:
                         phase1_qk(c, t)) for i in range(4)
                    ]
                phase2_ih(b, 0, fillers)
                if tiles_next is not None:
                    phase1_v((b + 1) * 4, tiles_next[0])
                    phase1_v((b + 1) * 4 + 1, tiles_next[1])
                phase2_ih(b, 1)
                if tiles_next is not None:
                    phase1_v((b + 1) * 4 + 2, tiles_next[2])
                    phase1_v((b + 1) * 4 + 3, tiles_next[3])
                phase3_batch(b)

    return nc


def _host_prep(x, key_padding_mask, dt, wq, wk, wv, wo):
    """Shared + per-core input arrays (all numpy)."""
    xT = np.ascontiguousarray(x.reshape(T, E).T).astype(ml_dtypes.bfloat16)

    # RoPE trig tables, rows [c;c;c;c] and [s;-s;s;-s] over 32-row blocks
    inv_freq = (1.0 / (THETA ** (np.arange(0, D, 2, dtype=np.float32) / D)))
    ang = dt.reshape(T).astype(np.float32)[None, :] * inv_freq[:, None]  # [32, T]
    cos = np.cos(ang).astype(np.float32)
    sin = np.sin(ang).astype(np.float32)
    cc = np.concatenate([cos, cos, cos, cos], axis=0)
    ssm = np.concatenate([sin, -sin, sin, -sin], axis=0)

    use_mask = bool(key_padding_mask.any())
    mb = None
    if use_mask:
        bias = np.where(key_padding_mask.reshape(T), NEG_INF, 0.0).astype(np.float32)
        # [128 j-in-block, B*16 block index]
        mb = np.ascontiguousarray(bias.reshape(B * 16, 128).T)

    # per-head channel permutation: [2r] then [2r+1] -> [r | 32+r]
    perm1 = np.concatenate([np.arange(0, D, 2), np.arange(1, D, 2)])

    per_core = []
    for c in range(NCORES):
        rows = []
        for h in range(c * HPC, (c + 1) * HPC):
            rows.append(h * D + perm1)
        rows = np.concatenate(rows)                      # permuted q/k rows
        rows_v = np.arange(c * CPC, (c + 1) * CPC)       # natural v rows
        # note: the 1/sqrt(D)=0.125 score scale is applied as the exp
        # activation's scale argument on device, not here
        wqT = np.ascontiguousarray(wq[rows].T).astype(ml_dtypes.bfloat16)
        wkT = np.ascontiguousarray(wk[rows].T).astype(ml_dtypes.bfloat16)
        wvT = np.ascontiguousarray(wv[rows_v].T).astype(ml_dtypes.bfloat16)
        woT = np.ascontiguousarray(wo[:, rows_v].T).astype(ml_dtypes.bfloat16)
        m = {"xT": xT, "cc": cc, "ss": ssm,
             "wqT": wqT, "wkT": wkT, "wvT": wvT, "woT": woT}
        if use_mask:
            m["mb"] = mb
        per_core.append(m)
    return per_core, use_mask


def kernel(x, key_padding_mask, dt, wq, wk, wv, wo, bo, _return_results=False):
    x = np.asarray(x, dtype=np.float32)
    key_padding_mask = np.asarray(key_padding_mask)
    dt = np.asarray(dt, dtype=np.float32)
    wq = np.asarray(wq, dtype=np.float32)
    wk = np.asarray(wk, dtype=np.float32)
    wv = np.asarray(wv, dtype=np.float32)
    wo = np.asarray(wo, dtype=np.float32)
    bo = np.asarray(bo, dtype=np.float32)

    in_maps, use_mask = _host_prep(x, key_padding_mask, dt, wq, wk, wv, wo)

    key = use_mask
    if key not in _prog_cache:
        prog = _build_program(use_mask)
        prog.finalize()
        _prog_cache[key] = prog
    nc = _prog_cache[key]

    res = run_bass_kernel_spmd(nc, in_maps, list(range(NCORES)))

    y = np.zeros((E, T), dtype=np.float32)
    for r in res.results:
        y += r["yT"]
    out = (y.T + bo[None, :]).reshape(B, S, E).astype(np.float32)
    if _return_results:
        return out, res
    return out



# revision 2
# speedup vs baseline: 1.3419x; 1.3419x over previous
"""MHA + RoPE fused kernel for Trainium2, sharded tensor-parallel over heads
across 8 NeuronCores.

Problem (hardcoded): B=4, S=2048, E=1024, H=16 heads, D=64.
  xq = x @ wq.T ; xk = x @ wk.T ; xv = x @ wv.T          [B,S,H,D]
  RoPE(xq, xk) with angles dt[b,s] * inv_freq[r]
  scores = softmax(xq @ xk.T / sqrt(D) + mask)            per (b, head)
  out = (scores @ xv) reshaped to [B,S,E]; y = out @ wo.T + bo

Sharding: each core owns 2 heads (128 channels of q/k/v) and the matching
128 rows of wo.T; it computes a full partial y (row-parallel output
projection) and the host sums the 8 partials (the "all-reduce" on host).

v2 layout: phase 2 runs per (batch, 512-query chunk) with a 16-beat j-loop:
  score-MM pair (2 heads row-packed) -> one [128,1024] exp (both heads)
  -> 2 AV matmuls accumulating into [65,512] pos banks (ones-column carries
  the softmax denominator).  QKV projections / RoPE / output projection are
  emitted as fine-grained filler thunks inside the beat loop so the PE never
  idles (keeps HAM at full clock) while ScalarE streams the exps.
PSUM: scores 2x[128,1024] (4 banks) + pos 2x[65,512] (2) + fillers (2) = 8.
"""

import sys

sys.path.insert(0, "/opt/trn_rl_repo")

from collections import deque

import numpy as np
import ml_dtypes

import concourse.bass as bass
from concourse import bacc
import concourse.tile as tile
from concourse import mybir
from concourse.bass_utils import run_bass_kernel_spmd

F32 = mybir.dt.float32
F16 = mybir.dt.float16
BF16 = mybir.dt.bfloat16

B, S, E, H, D = 4, 2048, 1024, 16, 64
T = B * S                      # 8192 flattened tokens
NCORES = 8
HPC = H // NCORES              # 2 heads per core
CPC = HPC * D                  # 128 channels per core
NCHUNK = T // 512              # 16 token chunks
KT = E // 128                  # 8 contraction tiles
THETA = 10000.0
NEG_INF = -1e30

_prog_cache = {}


def _build_program(use_mask: bool):
    """One Bass program, identical on every core (data differs per core)."""
    nc = bacc.Bacc()

    xT_d = nc.dram_tensor("xT", [E, T], BF16, kind="ExternalInput")
    cc_d = nc.dram_tensor("cc", [128, T], F16, kind="ExternalInput")
    ss_d = nc.dram_tensor("ss", [128, T], F16, kind="ExternalInput")
    wq_d = nc.dram_tensor("wqT", [E, CPC], BF16, kind="ExternalInput")
    wk_d = nc.dram_tensor("wkT", [E, CPC], BF16, kind="ExternalInput")
    wv_d = nc.dram_tensor("wvT", [E, CPC], BF16, kind="ExternalInput")
    wo_d = nc.dram_tensor("woT", [CPC, E], BF16, kind="ExternalInput")
    scr_d = nc.dram_tensor("csscr", [B * 8, 512], F32)  # colsum-recip bounce
    mb_d = None
    if use_mask:
        mb_d = nc.dram_tensor("mb", [128, B * 16], F32, kind="ExternalInput")
    y_d = nc.dram_tensor("yT", [E, T], F16, kind="ExternalOutput")

    xT_r = xT_d.rearrange("(k p) t -> p k t", p=128)
    wq_r = wq_d.rearrange("(k p) c -> p k c", p=128)
    wk_r = wk_d.rearrange("(k p) c -> p k c", p=128)
    wv_r = wv_d.rearrange("(k p) c -> p k c", p=128)
    wo_r = wo_d.rearrange("p (k c) -> p k c", c=128)

    with tile.TileContext(nc) as tc:
        with (
            tc.tile_pool(name="consts", bufs=1) as consts,
            tc.tile_pool(name="big", bufs=1) as big,
            tc.tile_pool(name="ph1", bufs=4) as ph1,
            tc.tile_pool(name="rope", bufs=2) as rope,
            tc.tile_pool(name="pt", bufs=3) as ptp,
            tc.tile_pool(name="norm", bufs=2) as norm,
            tc.tile_pool(name="ph3", bufs=3) as ph3,
            tc.tile_pool(name="psS", bufs=2, space="PSUM") as psS,
            tc.tile_pool(name="psP", bufs=2, space="PSUM") as psP,
            tc.tile_pool(name="psF", bufs=2, space="PSUM") as psF,
        ):
            # ---- constants ----
            wq_sb = consts.tile([128, KT, CPC], BF16)
            wk_sb = consts.tile([128, KT, CPC], BF16)
            wv_sb = consts.tile([128, KT, CPC], BF16)
            wo_sb = consts.tile([128, KT, 128], BF16)
            nc.sync.dma_start(wq_sb, wq_r)
            nc.sync.dma_start(wk_sb, wk_r)
            nc.sync.dma_start(wv_sb, wv_r)
            nc.sync.dma_start(wo_sb, wo_r)
            mb_sb = None
            if use_mask:
                mb_sb = consts.tile([128, B * 16], F32)
                nc.sync.dma_start(mb_sb, mb_d[:, :])

            # ---- persistent activations ----
            qT_sb = big.tile([128, NCHUNK, 512], BF16)
            kT_sb = big.tile([128, NCHUNK, 512], BF16)
            vA_sb = big.tile([128, T // 128, 65], BF16)
            vB_sb = big.tile([128, T // 128, 65], BF16)
            attnT_sb = big.tile([128, NCHUNK, 512], BF16)
            nc.vector.memset(vA_sb[:, :, 64], 1.0)
            nc.vector.memset(vB_sb[:, :, 64], 1.0)

            def phase1_load(ch):
                """Issue the DMAs for token chunk ch; returns the tiles."""
                xsb = ph1.tile([128, KT, 512], BF16, tag="xsb")
                for k in range(KT):   # one DMA per k-tile -> spread queues
                    nc.sync.dma_start(xsb[:, k, :],
                                      xT_r[:, k, ch * 512:(ch + 1) * 512])
                cc_sb = ph1.tile([128, 512], F16, tag="cc")
                ss_sb = ph1.tile([128, 512], F16, tag="ss")
                nc.sync.dma_start(cc_sb, cc_d[:, ch * 512:(ch + 1) * 512])
                nc.sync.dma_start(ss_sb, ss_d[:, ch * 512:(ch + 1) * 512])
                return xsb, cc_sb, ss_sb

            def gen_qk(ch, tiles, w_sb, dstT):
                """Micro-steps of one q-or-k projection + RoPE for chunk ch."""
                xsb, cc_sb, ss_sb = tiles
                st = {}

                def mm(k0):
                    if k0 == 0:
                        st["ps"] = psF.tile([128, 512], F32, tag="fill",
                                            name="ps_qk")
                    for k in range(k0, k0 + 4):
                        nc.tensor.matmul(st["ps"], w_sb[:, k, :], xsb[:, k, :],
                                         start=(k == 0), stop=(k == KT - 1))

                def rope():
                    ps = st["ps"]
                    t1 = rope_pool.tile([128, 512], F32, tag="t1")
                    t2 = rope_pool.tile([128, 512], F32, tag="t2")
                    t2sw = rope_pool.tile([128, 512], F32, tag="t2sw")
                    nc.vector.tensor_tensor(t1, ps, cc_sb, mybir.AluOpType.mult)
                    nc.vector.tensor_tensor(t2, ps, ss_sb, mybir.AluOpType.mult)
                    for b0 in (0, 64):
                        nc.sync.dma_start(t2sw[b0:b0 + 32], t2[b0 + 32:b0 + 64])
                        nc.sync.dma_start(t2sw[b0 + 32:b0 + 64], t2[b0:b0 + 32])
                    nc.vector.tensor_tensor(dstT[:, ch, :], t1, t2sw,
                                            mybir.AluOpType.add)

                yield from (lambda: mm(0), lambda: mm(4), rope)

            def gen_v(ch, tiles):
                """Micro-steps of the v projection for chunk ch."""
                xsb, _, _ = tiles
                st = {}

                def mm(tt, k0):
                    if k0 == 0:
                        st["psv"] = psF.tile([128, 128], F32, tag="fill",
                                             name="psv")
                    for k in range(k0, k0 + 4):
                        nc.tensor.matmul(st["psv"],
                                         xsb[:, k, tt * 128:(tt + 1) * 128],
                                         wv_sb[:, k, :],
                                         start=(k == 0), stop=(k == KT - 1))
                    if k0 == 4:
                        ti = ch * 4 + tt
                        nc.vector.tensor_copy(vA_sb[:, ti, 0:64],
                                              st["psv"][:, 0:64])
                        nc.vector.tensor_copy(vB_sb[:, ti, 0:64],
                                              st["psv"][:, 64:128])

                for tt in range(4):
                    yield (lambda tt=tt: mm(tt, 0))
                    yield (lambda tt=tt: mm(tt, 4))

            def gen_out(b, cck):
                """Micro-steps of output projection rows cck for batch b."""
                def step(tc2):
                    for tc4 in (tc2, tc2 + 1):
                        ch = b * 4 + tc4
                        psy = psF.tile([128, 512], F32, tag="fill", name="psy")
                        nc.tensor.matmul(psy, wo_sb[:, cck, :],
                                         attnT_sb[:, ch, :],
                                         start=True, stop=True)
                        ysb = ph3.tile([128, 512], F16, tag="ysb")
                        nc.vector.tensor_copy(ysb, psy)
                        nc.sync.dma_start(
                            y_d[cck * 128:(cck + 1) * 128,
                                ch * 512:(ch + 1) * 512], ysb)

                yield (lambda: step(0))
                yield (lambda: step(2))

            rope_pool = rope

            def phase2_qc(b, qc, fillers):
                """Attention for batch b, query chunk qc (512 queries)."""
                ch_i = b * 4 + qc
                pos = [psP.tile([65, 512], F32, tag="pos", name=f"pos{_h}")
                       for _h in range(2)]
                for jb in range(16):
                    pss = psS.tile([128, 1024], F32, tag="sc", name="pss")
                    ch_j = b * 4 + jb // 4
                    off_j = (jb % 4) * 128
                    for hh, b0 in ((0, 0), (1, 64)):
                        nc.tensor.matmul(
                            pss[:, hh * 512:(hh + 1) * 512],
                            kT_sb[b0:b0 + 64, ch_j, off_j:off_j + 128],
                            qT_sb[b0:b0 + 64, ch_i, :],
                            start=True, stop=True,
                            tile_position=(b0, 0))
                    # filler thunks slot into the exp-latency gap on the PE
                    for _ in range(2):
                        if fillers:
                            fillers.popleft()()
                    pT = ptp.tile([128, 1024], BF16, tag="pT", name="pT")
                    bias = (mb_sb[:, b * 16 + jb:b * 16 + jb + 1]
                            if use_mask else 0.0)
                    nc.scalar.activation(pT, pss,
                                         mybir.ActivationFunctionType.Exp,
                                         bias=bias, scale=0.125)
                    for hh, v_sb in ((0, vA_sb), (1, vB_sb)):
                        nc.tensor.matmul(
                            pos[hh],
                            v_sb[:, b * 16 + jb, :],
                            pT[:, hh * 512:(hh + 1) * 512],
                            start=(jb == 0), stop=(jb == 15))
                for hh in range(2):
                    csrow = norm.tile([1, 512], F32, tag="csrow")
                    nc.vector.tensor_copy(csrow, pos[hh][64:65, :])
                    cs_rec = norm.tile([1, 512], F32, tag="csrec")
                    nc.vector.reciprocal_approx_fast(out=cs_rec, in_=csrow)
                    srow = (b * 4 + qc) * 2 + hh
                    nc.sync.dma_start(scr_d[srow:srow + 1, :], cs_rec)
                    csrep = norm.tile([64, 512], F32, tag="csrep")
                    nc.gpsimd.dma_start(
                        csrep, scr_d[srow, :].partition_broadcast(64))
                    ocp = norm.tile([64, 512], F32, tag="ocp")
                    nc.vector.tensor_copy(ocp, pos[hh][0:64, :])
                    nc.vector.tensor_tensor(
                        attnT_sb[hh * 64:hh * 64 + 64, ch_i, :],
                        ocp, csrep, mybir.AluOpType.mult)

            # ---- schedule ----
            # batch 0's inputs: load + project up front
            tiles0 = [phase1_load(ch) for ch in range(4)]
            warm = deque()
            for ch in range(4):
                warm.extend(gen_qk(ch, tiles0[ch], wq_sb, qT_sb))
                warm.extend(gen_qk(ch, tiles0[ch], wk_sb, kT_sb))
                warm.extend(gen_v(ch, tiles0[ch]))
            while warm:
                warm.popleft()()

            for b in range(B):
                fillers = deque()
                if b + 1 < B:
                    tiles_next = [phase1_load((b + 1) * 4 + i)
                                  for i in range(4)]
                    for i in range(4):
                        c = (b + 1) * 4 + i
                        fillers.extend(gen_qk(c, tiles_next[i], wq_sb, qT_sb))
                        fillers.extend(gen_qk(c, tiles_next[i], wk_sb, kT_sb))
                        fillers.extend(gen_v(c, tiles_next[i]))
                if b > 0:
                    for cck in range(KT):
                        fillers.extend(gen_out(b - 1, cck))
                for qc in range(4):
                    phase2_qc(b, qc, fillers)
                while fillers:
                    fillers.popleft()()
            # tail: output projection of the last batch
            tailf = deque()
            for cck in range(KT):
                tailf.extend(gen_out(B - 1, cck))
            while tailf:
                tailf.popleft()()

    return nc


def _host_prep(x, key_padding_mask, dt, wq, wk, wv, wo):
    """Shared + per-core input arrays (all numpy)."""
    xT = np.ascontiguousarray(x.reshape(T, E).T).astype(ml_dtypes.bfloat16)

    # RoPE trig tables, rows [c;c;c;c] and [s;-s;s;-s] over 32-row blocks
    inv_freq = (1.0 / (THETA ** (np.arange(0, D, 2, dtype=np.float32) / D)))
    ang = dt.reshape(T).astype(np.float32)[None, :] * inv_freq[:, None]  # [32, T]
    cos = np.cos(ang).astype(np.float32)
    sin = np.sin(ang).astype(np.float32)
    cc = np.concatenate([cos, cos, cos, cos], axis=0).astype(np.float16)
    ssm = np.concatenate([sin, -sin, sin, -sin], axis=0).astype(np.float16)

    use_mask = bool(key_padding_mask.any())
    mb = None
    if use_mask:
        bias = np.where(key_padding_mask.reshape(T), NEG_INF, 0.0).astype(np.float32)
        # [128 j-in-block, B*16 block index]
        mb = np.ascontiguousarray(bias.reshape(B * 16, 128).T)

    # per-head channel permutation: [2r] then [2r+1] -> [r | 32+r]
    perm1 = np.concatenate([np.arange(0, D, 2), np.arange(1, D, 2)])

    per_core = []
    for c in range(NCORES):
        rows = []
        for h in range(c * HPC, (c + 1) * HPC):
            rows.append(h * D + perm1)
        rows = np.concatenate(rows)                      # permuted q/k rows
        rows_v = np.arange(c * CPC, (c + 1) * CPC)       # natural v rows
        # note: the 1/sqrt(D)=0.125 score scale is applied as the exp
        # activation's scale argument on device, not here
        wqT = np.ascontiguousarray(wq[rows].T).astype(ml_dtypes.bfloat16)
        wkT = np.ascontiguousarray(wk[rows].T).astype(ml_dtypes.bfloat16)
        wvT = np.ascontiguousarray(wv[rows_v].T).astype(ml_dtypes.bfloat16)
        woT = np.ascontiguousarray(wo[:, rows_v].T).astype(ml_dtypes.bfloat16)
        m = {"xT": xT, "cc": cc, "ss": ssm,
             "wqT": wqT, "wkT": wkT, "wvT": wvT, "woT": woT}
        if use_mask:
            m["mb"] = mb
        per_core.append(m)
    return per_core, use_mask


def kernel(x, key_padding_mask, dt, wq, wk, wv, wo, bo, _return_results=False):
    x = np.asarray(x, dtype=np.float32)
    key_padding_mask = np.asarray(key_padding_mask)
    dt = np.asarray(dt, dtype=np.float32)
    wq = np.asarray(wq, dtype=np.float32)
    wk = np.asarray(wk, dtype=np.float32)
    wv = np.asarray(wv, dtype=np.float32)
    wo = np.asarray(wo, dtype=np.float32)
    bo = np.asarray(bo, dtype=np.float32)

    in_maps, use_mask = _host_prep(x, key_padding_mask, dt, wq, wk, wv, wo)

    key = use_mask
    if key not in _prog_cache:
        prog = _build_program(use_mask)
        prog.finalize()
        _prog_cache[key] = prog
    nc = _prog_cache[key]

    res = run_bass_kernel_spmd(nc, in_maps, list(range(NCORES)))

    y = np.zeros((E, T), dtype=np.float32)
    for r in res.results:
        y += r["yT"].astype(np.float32)
    out = (y.T + bo[None, :]).reshape(B, S, E).astype(np.float32)
    if _return_results:
        return out, res
    return out
